# revision 7
# baseline (speedup 1.0000x reference)
"""Trainium2 Bass kernel for nn_Decoder_39591008535099 (social-GAN style decoder).

Strategy
--------
Data-parallel over pedestrian groups: 8 NeuronCores, each owns 32 groups
(512 pedestrians). All weights replicated. Everything is computed in a
"transposed" layout: features on SBUF partitions, pedestrians on the free
axis, so BatchNorm statistics are free-axis reductions and matmuls chain as
out = W.T @ actT without any transposes.

Pool-net algebra (exact):
  Y1[a,b] = emb(pos_b - pos_a) @ Wp1a + h_b @ Wp1b + const
          = u[b] - v[a] + const,   u = pe@Wp1a + h@Wp1b, v = pe@Wp1a
  Per-group BN1 over the P*P grid: mean/var separate into u/v moments
  (Var = Var_b(u) + Var_a(v)); biases/embedding constants cancel in the
  centering, so they are dropped exactly.  X2 = relu(s*(u_c[b] - v_c[a]) + beta).
  BN2-then-max over b commutes with max since gamma2 > 0 (monotone affine),
  so the P*P tensor is reduced by max over raw Y2 first.

Merge-MLP BatchNorm needs exact global batch stats: each core computes
bn_stats partials, one 6KB AllGather per layer merges them (bn_aggr), which
is bit-faithful Welford pooling.  (Local per-shard stats diverge
catastrophically: this model amplifies per-step error ~5e4x over 12 steps.)

All matmuls f32 (bf16 is numerically fatal here), rsqrt is Newton-refined.
"""
import os
import numpy as np

try:
    import concourse.bass as bass
except ImportError:  # pragma: no cover
    import sys
    sys.path.insert(0, "/opt/trn_rl_repo")
    import concourse.bass as bass
import concourse.bacc as bacc
import concourse.tile as tile
import concourse.mybir as mybir
import concourse.bass_utils as bass_utils

AF = mybir.ActivationFunctionType
OP = mybir.AluOpType
AX = mybir.AxisListType
F32 = mybir.dt.float32

NCORES = 8
T_STEPS = int(os.environ.get("KERNEL_T_STEPS", "12"))
R = 512          # pedestrians per core
GS = 32          # groups per core
P = 16           # peds per group
EPS = 1e-5
SIG_MODE = os.environ.get("KERNEL_SIG_MODE", "native")   # "native" | "exp"

_CACHE = {}


def _nr_rsqrt(nc, wp, vraw, scale, name, rstd_mode="sqrt"):
    """rstd = 1/sqrt(vraw*scale + EPS), Newton-refined to ~1ulp.
    vraw: (p, n) AP. Returns SBUF tile (p, n)."""
    pdim, n = vraw.shape[0], vraw.shape[1] if len(vraw.shape) == 2 else None
    assert n is not None
    t = wp.tile([pdim, n], F32, name=f"{name}_t")
    nc.vector.tensor_scalar(out=t[:], in0=vraw, scalar1=float(scale),
                            scalar2=float(EPS), op0=OP.mult, op1=OP.add)
    r0 = wp.tile([pdim, n], F32, name=f"{name}_r0")
    if rstd_mode == "expln":
        l = wp.tile([pdim, n], F32, name=f"{name}_l")
        nc.scalar.activation(l[:], t[:], AF.Ln)
        nc.scalar.activation(r0[:], l[:], AF.Exp, scale=-0.5)
    else:
        sq = wp.tile([pdim, n], F32, name=f"{name}_sq")
        nc.scalar.activation(sq[:], t[:], AF.Sqrt)
        scr = wp.tile([pdim, n], F32, name=f"{name}_scr")
        nc.vector.reciprocal_approx_accurate(r0[:], sq[:], scr[:])
    # two Newton steps for rsqrt: r <- r*(1.5 - 0.5*t*r^2)
    r = r0
    for it in range(2):
        r2 = wp.tile([pdim, n], F32, name=f"{name}_r2{it}")
        nc.vector.tensor_mul(r2[:], r[:], r[:])
        tr = wp.tile([pdim, n], F32, name=f"{name}_tr{it}")
        nc.vector.tensor_mul(tr[:], t[:], r2[:])
        w = wp.tile([pdim, n], F32, name=f"{name}_w{it}")
        nc.vector.tensor_scalar(out=w[:], in0=tr[:], scalar1=-0.5, scalar2=1.5,
                                op0=OP.mult, op1=OP.add)
        rn = wp.tile([pdim, n], F32, name=f"{name}_rn{it}")
        nc.vector.tensor_mul(rn[:], r[:], w[:])
        r = rn
    return r


def _emit(nc, tc, d):
    RG = [list(range(NCORES))]
    with tc.tile_pool(name="const", bufs=1) as cp, \
         tc.tile_pool(name="work", bufs=2) as wp, \
         tc.tile_pool(name="psum", bufs=7, space="PSUM") as pp, \
         tc.tile_pool(name="dram", bufs=2, space="DRAM") as dp:

        def load(name, shape):
            tl = cp.tile(list(shape), F32, name=f"w_{name}")
            nc.sync.dma_start(tl[:], d[name].ap())
            return tl

        wih = load("Wih", (64, 512))
        whh = load("Whh", (128, 512))
        wse = load("Wse", (2, 64))
        whp = load("Whp", (128, 2))
        wpse = load("Wpse", (2, 64))
        wp1a = load("Wp1a", (64, 512))
        wp1b = load("Wp1b", (128, 512))
        wp2 = load("Wp2r", (128, 512))
        wm1 = load("Wm1r", (128, 2048))
        wm2 = load("Wm2r", (128, 1024))
        bgS = load("bgS", (128, 4))
        bgnS = load("bgnS", (128, 4))
        bg2S = load("bg2S", (128, 4))
        bseS = load("bseS", (64, 1))
        bhpS = load("bhpS", (2, 1))
        gp1S = load("gp1S", (128, 4))
        bp1S = load("bp1S", (128, 4))
        gp2S = load("gp2S", (128, 1))
        bp2S = load("bp2S", (128, 1))
        gm1S = load("gm1S", (128, 8))
        bm1S = load("bm1S", (128, 8))
        gm2S = load("gm2S", (128, 1))
        bm2S = load("bm2S", (128, 1))

        hT = load("hT", (128, 512))
        cT = load("cT", (128, 512))
        posT = load("posT", (2, 512))
        relpT = load("relposT", (2, 512))

        # initial dec_in = relpos @ Wse + bse
        decT = cp.tile([64, 512], F32, name="decT")
        pd0 = pp.tile([128, 512], F32, name="ps", tag="ps")
        nc.tensor.matmul(pd0[0:64, :], wse[:], relpT[:], start=True, stop=True)
        nc.vector.tensor_scalar_add(decT[:], pd0[0:64, :], bseS[:, 0:1])

        def act_sigmoid(dst, src, bias_col, nbias_col):
            if SIG_MODE == "native":
                nc.scalar.activation(dst[:], src, AF.Sigmoid, bias=bias_col, scale=1.0)
            else:
                e = wp.tile([128, 512], F32, name="sig_e")
                nc.scalar.activation(e[:], src, AF.Exp, bias=nbias_col, scale=-1.0)
                nc.vector.tensor_scalar_add(e[:], e[:], 1.0)
                scr = wp.tile([128, 512], F32, name="sig_scr")
                nc.vector.reciprocal_approx_accurate(dst[:], e[:], scr[:])

        def act_tanh(dst, src, bias_col, bias2_col):
            if SIG_MODE == "native":
                if bias_col is None:
                    nc.scalar.activation(dst[:], src, AF.Tanh)
                else:
                    nc.scalar.activation(dst[:], src, AF.Tanh, bias=bias_col, scale=1.0)
            else:
                e = wp.tile([128, 512], F32, name="tanh_e")
                if bias2_col is None:
                    nc.scalar.activation(e[:], src, AF.Exp, scale=2.0)
                else:
                    nc.scalar.activation(e[:], src, AF.Exp, bias=bias2_col, scale=2.0)
                nc.vector.tensor_scalar_add(e[:], e[:], 1.0)
                scr = wp.tile([128, 512], F32, name="tanh_scr")
                rec = wp.tile([128, 512], F32, name="tanh_rec")
                nc.vector.reciprocal_approx_accurate(rec[:], e[:], scr[:])
                nc.vector.tensor_scalar(out=dst[:], in0=rec[:], scalar1=-2.0,
                                        scalar2=1.0, op0=OP.mult, op1=OP.add)

        warm_ctr = [0]

        def keep_warm(n):
            pdum = pp.tile([128, 512], F32, name="pdum", tag="pdum", bufs=1)
            for i in range(n):
                nc.tensor.matmul(pdum[:], wm2[:, 0:128], wm2[:, 0:512],
                                 start=(i == 0), stop=(i == n - 1))
            nc.vector.tensor_copy(warm_sink[:], pdum[:, 0:1])
            warm_ctr[0] += n

        warm_sink = cp.tile([128, 1], F32, name="warm_sink")

        for t in range(T_STEPS):
            # ============ LSTM cell ============
            gact = []
            for j in range(4):
                pg = pp.tile([128, 512], F32, name="ps", tag="ps")
                nc.tensor.matmul(pg[:], wih[:, 128 * j:128 * (j + 1)], decT[:],
                                 start=True, stop=False)
                nc.tensor.matmul(pg[:], whh[:, 128 * j:128 * (j + 1)], hT[:],
                                 start=False, stop=True)
                g_t = wp.tile([128, 512], F32, name=f"gact{j}", bufs=1)
                if j == 2:
                    act_tanh(g_t, pg[:], bgS[:, j:j + 1], bg2S[:, j:j + 1])
                else:
                    act_sigmoid(g_t, pg[:], bgS[:, j:j + 1], bgnS[:, j:j + 1])
                gact.append(g_t)
            keep_warm(10)
            t1 = wp.tile([128, 512], F32, name="lstm_t1")
            nc.vector.tensor_mul(t1[:], gact[1][:], cT[:])
            t2 = wp.tile([128, 512], F32, name="lstm_t2")
            nc.vector.tensor_mul(t2[:], gact[0][:], gact[2][:])
            nc.vector.tensor_add(cT[:], t1[:], t2[:])
            tcn = wp.tile([128, 512], F32, name="lstm_tc")
            act_tanh(tcn, cT[:], None, None)
            hn = wp.tile([128, 512], F32, name="hn")
            nc.vector.tensor_mul(hn[:], gact[3][:], tcn[:])

            # ============ rel_pos / pos / traj / dec_in ============
            prp = pp.tile([128, 512], F32, name="ps", tag="ps")
            nc.tensor.matmul(prp[0:2, :], whp[:], hn[:], start=True, stop=True)
            relT = wp.tile([2, 512], F32, name="relT")
            nc.vector.tensor_scalar_add(relT[:], prp[0:2, :], bhpS[:, 0:1])
            nc.vector.tensor_add(posT[:], posT[:], relT[:])
            nc.sync.dma_start(d["traj"].ap()[t], relT[:])

            pdi = pp.tile([128, 512], F32, name="ps", tag="ps")
            nc.tensor.matmul(pdi[0:64, :], wse[:], relT[:], start=True, stop=True)
            nc.vector.tensor_scalar_add(decT[:], pdi[0:64, :], bseS[:, 0:1])

            # ============ pool: pe + input means ============
            ppe = pp.tile([128, 512], F32, name="ps", tag="ps")
            nc.tensor.matmul(ppe[0:64, :], wpse[:], posT[:], start=True, stop=True)
            peT = wp.tile([64, 512], F32, name="peT")
            nc.scalar.copy(peT[:], ppe[0:64, :])

            pe_m = wp.tile([64, 32], F32, name="pe_m")
            nc.vector.reduce_sum(pe_m[:],
                                 peT[:].rearrange("p (g b) -> p g b", b=P), axis=AX.X)
            pe_ne = wp.tile([64, 512], F32, name="pe_ne")
            nc.vector.tensor_scalar_mul(
                pe_ne[:].rearrange("p (g b) -> p g b", b=P),
                pe_m[:].unsqueeze(2).broadcast_to((64, GS, P)), -1.0 / P)
            h_m = wp.tile([128, 32], F32, name="h_m")
            nc.vector.reduce_sum(h_m[:],
                                 hn[:].rearrange("p (g b) -> p g b", b=P), axis=AX.X)
            h_ne = wp.tile([128, 512], F32, name="h_ne")
            nc.vector.tensor_scalar_mul(
                h_ne[:].rearrange("p (g b) -> p g b", b=P),
                h_m[:].unsqueeze(2).broadcast_to((128, GS, P)), -1.0 / P)

            # ============ pool: centered v and u, variances ============
            vcS = wp.tile([128, 2048], F32, name="vcS", bufs=1)
            ucS = wp.tile([128, 2048], F32, name="ucS", bufs=1)
            sqv = wp.tile([128, 2048], F32, name="sqv", bufs=1)
            squ = wp.tile([128, 2048], F32, name="squ", bufs=1)
            for j in range(4):
                jsl = slice(128 * j, 128 * (j + 1))
                pv = pp.tile([128, 512], F32, name="ps", tag="ps")
                nc.tensor.matmul(pv[:], wp1a[:, jsl], peT[:], start=True, stop=False)
                nc.tensor.matmul(pv[:], wp1a[:, jsl], pe_ne[:], start=False, stop=True)
                nc.scalar.copy(vcS[:, 512 * j:512 * (j + 1)], pv[:])
                nc.scalar.square(sqv[:, 512 * j:512 * (j + 1)], pv[:])
            for j in range(4):
                jsl = slice(128 * j, 128 * (j + 1))
                pu = pp.tile([128, 512], F32, name="ps", tag="ps")
                nc.tensor.matmul(pu[:], wp1b[:, jsl], hn[:], start=True, stop=False)
                nc.tensor.matmul(pu[:], wp1b[:, jsl], h_ne[:], start=False, stop=False)
                nc.tensor.matmul(pu[:], wp1a[:, jsl], peT[:], start=False, stop=False)
                nc.tensor.matmul(pu[:], wp1a[:, jsl], pe_ne[:], start=False, stop=True)
                nc.scalar.copy(ucS[:, 512 * j:512 * (j + 1)], pu[:])
                nc.scalar.square(squ[:, 512 * j:512 * (j + 1)], pu[:])

            vss = wp.tile([128, 128], F32, name="vss")
            nc.vector.reduce_sum(
                vss[:].rearrange("p (c g) -> p c g", g=GS),
                sqv[:].rearrange("p (c g b) -> p c g b", g=GS, b=P), axis=AX.X)
            uss = wp.tile([128, 128], F32, name="uss")
            nc.vector.reduce_sum(
                uss[:].rearrange("p (c g) -> p c g", g=GS),
                squ[:].rearrange("p (c g b) -> p c g b", g=GS, b=P), axis=AX.X)

            vv = wp.tile([128, 128], F32, name="vv")
            nc.vector.tensor_add(vv[:], uss[:], vss[:])
            rstd1 = _nr_rsqrt(nc, wp, vv[:], 1.0 / P, "rs1")
            s1 = wp.tile([128, 128], F32, name="s1")
            for j in range(4):
                nc.vector.tensor_scalar_mul(s1[:, 32 * j:32 * (j + 1)],
                                            rstd1[:, 32 * j:32 * (j + 1)],
                                            gp1S[:, j:j + 1])
            s1e = wp.tile([128, 2048], F32, name="sqv", bufs=1)
            nc.vector.tensor_copy(
                s1e[:].rearrange("p (c g b) -> p c g b", g=GS, b=P),
                s1[:].rearrange("p (c g) -> p c g", g=GS)
                    .unsqueeze(3).broadcast_to((128, 4, GS, P)))
            ucp = ucS
            nc.vector.tensor_mul(ucp[:], ucS[:], s1e[:])
            for j in range(4):
                nc.vector.tensor_scalar_add(ucp[:, 512 * j:512 * (j + 1)],
                                            ucp[:, 512 * j:512 * (j + 1)],
                                            bp1S[:, j:j + 1])
            vcp = vcS
            nc.vector.tensor_mul(vcp[:], vcS[:], s1e[:])

            # ============ pool: per-quad X2 -> Y2 -> stats/max ============
            poolmax = wp.tile([128, 512], F32, name="poolmax", bufs=1)
            mstY = wp.tile([128, 192], F32, name="mstY", bufs=1)
            for q in range(8):
                x2t = []
                for j in range(4):
                    x2 = wp.tile([128, 1024], F32, name=f"x2_{j}", bufs=2)
                    base = 512 * j + 64 * q
                    u_ap = (ucp[:, base:base + 64]
                            .rearrange("p (g b) -> p g b", b=P)
                            .unsqueeze(2).broadcast_to((128, 4, P, P)))
                    v_ap = (vcp[:, base:base + 64]
                            .rearrange("p (g a) -> p g a", a=P)
                            .unsqueeze(3).broadcast_to((128, 4, P, P)))
                    if j == 3:
                        nc.gpsimd.tensor_sub(
                            x2[:].rearrange("p (g a b) -> p g a b", a=P, b=P),
                            u_ap, v_ap)
                    else:
                        nc.vector.tensor_sub(
                            x2[:].rearrange("p (g a b) -> p g a b", a=P, b=P),
                            u_ap, v_ap)
                    if j < 2:
                        nc.scalar.activation(x2[:], x2[:], AF.Relu)
                    else:
                        nc.vector.tensor_scalar_max(x2[:], x2[:], 0.0)
                    x2t.append(x2)
                for h2 in range(2):
                    py2 = pp.tile([128, 512], F32, name="ps", tag="ps")
                    for j in range(4):
                        nc.tensor.matmul(py2[:], wp2[:, 128 * j:128 * (j + 1)],
                                         x2t[j][:, 512 * h2:512 * (h2 + 1)],
                                         start=(j == 0), stop=(j == 3))
                    gbase = 4 * q + 2 * h2
                    for g2 in range(2):
                        nc.vector.bn_stats(
                            mstY[:, 6 * (gbase + g2):6 * (gbase + g2) + 6],
                            py2[:, 256 * g2:256 * (g2 + 1)])
                    nc.vector.reduce_max(
                        poolmax[:, 64 * q + 32 * h2:64 * q + 32 * (h2 + 1)]
                        .rearrange("p (g a) -> p g a", a=P),
                        py2[:].rearrange("p (g a b) -> p g a b", a=P, b=P),
                        axis=AX.X)

            # ============ pool BN2 (monotone max trick) ============
            mr = mstY[:].rearrange("p (g s) -> p g s", s=6)
            dm = wp.tile([128, 32], F32, name="dm")
            nc.vector.tensor_sub(dm[:], mr[:, :, 1], mr[:, :, 4])
            dm2 = wp.tile([128, 32], F32, name="dm2")
            nc.vector.tensor_mul(dm2[:], dm[:], dm[:])
            cv = wp.tile([128, 32], F32, name="cvt")
            nc.vector.tensor_add(cv[:], mr[:, :, 2], mr[:, :, 5])
            v256 = wp.tile([128, 32], F32, name="v256")
            nc.vector.scalar_tensor_tensor(out=v256[:], in0=dm2[:], scalar=64.0,
                                           in1=cv[:], op0=OP.mult, op1=OP.add)
            msum = wp.tile([128, 32], F32, name="msum")
            nc.vector.tensor_add(msum[:], mr[:, :, 1], mr[:, :, 4])
            rstd2 = _nr_rsqrt(nc, wp, v256[:], 1.0 / 256.0, "rs2")
            s2 = wp.tile([128, 32], F32, name="s2")
            nc.vector.tensor_scalar_mul(s2[:], rstd2[:], gp2S[:, 0:1])
            ms2 = wp.tile([128, 32], F32, name="ms2")
            nc.vector.tensor_mul(ms2[:], msum[:], s2[:])
            ms2b = wp.tile([128, 32], F32, name="ms2b")
            nc.vector.tensor_scalar(out=ms2b[:], in0=ms2[:], scalar1=0.5,
                                    scalar2=bp2S[:, 0:1], op0=OP.mult,
                                    op1=OP.subtract)
            s2e = wp.tile([128, 512], F32, name="s2e", bufs=1)
            nc.vector.tensor_copy(
                s2e[:].rearrange("p (g b) -> p g b", b=P),
                s2[:].unsqueeze(2).broadcast_to((128, GS, P)))
            mbe = wp.tile([128, 512], F32, name="mbe", bufs=1)
            nc.vector.tensor_copy(
                mbe[:].rearrange("p (g b) -> p g b", b=P),
                ms2b[:].unsqueeze(2).broadcast_to((128, GS, P)))
            pms = wp.tile([128, 512], F32, name="pms", bufs=1)
            nc.vector.tensor_mul(pms[:], poolmax[:], s2e[:])
            poolT = wp.tile([128, 512], F32, name="poolT", bufs=1)
            nc.vector.tensor_sub(poolT[:], pms[:], mbe[:])
            nc.scalar.activation(poolT[:], poolT[:], AF.Relu)

            # ============ merge layer 1 + global BN (AllGather) ============
            ym = wp.tile([128, 4096], F32, name="ym", bufs=1)
            mst1 = wp.tile([128, 48], F32, name="mst1")
            for j in range(8):
                pm1 = pp.tile([128, 512], F32, name="ps", tag="ps")
                nc.tensor.matmul(pm1[:], wm1[:, 128 * j:128 * (j + 1)], hn[:],
                                 start=True, stop=False)
                nc.tensor.matmul(pm1[:], wm1[:, 1024 + 128 * j:1024 + 128 * (j + 1)],
                                 poolT[:], start=False, stop=True)
                nc.scalar.copy(ym[:, 512 * j:512 * (j + 1)], pm1[:])
                nc.vector.bn_stats(mst1[:, 6 * j:6 * j + 6],
                                   ym[:, 512 * j:512 * (j + 1)])
            b1i = dp.tile([128, 48], F32, name="b1i")
            nc.sync.dma_start(b1i[:], mst1[:])
            b1o = dp.tile([1024, 48], F32, name="b1o")
            nc.gpsimd.collective_compute("AllGather", OP.bypass, replica_groups=RG,
                                         ins=[b1i.opt()], outs=[b1o.opt()])
            keep_warm(30)
            gst1 = wp.tile([128, 384], F32, name="gst1")
            nc.sync.dma_start(gst1[:].rearrange("p (r s) -> p r s", r=8),
                              b1o[:].rearrange("(r p) s -> p r s", p=128))
            mv1 = wp.tile([128, 16], F32, name="mv1")
            g1r = gst1[:].rearrange("p (r c t e) -> p r c t e", r=8, c=8, e=3)
            for j in range(8):
                nc.vector.bn_aggr(mv1[:, 2 * j:2 * j + 2], g1r[:, :, j])
            mv1r = mv1[:].rearrange("p (j k) -> p j k", k=2)
            rstd1m = _nr_rsqrt(nc, wp, mv1r[:, :, 1], 1.0, "rm1")
            s1m = wp.tile([128, 8], F32, name="s1m")
            nc.vector.tensor_mul(s1m[:], rstd1m[:], gm1S[:])
            t1m = wp.tile([128, 8], F32, name="t1m")
            nc.vector.tensor_mul(t1m[:], mv1r[:, :, 0], s1m[:])
            b1m = wp.tile([128, 8], F32, name="b1m")
            nc.vector.tensor_sub(b1m[:], bm1S[:], t1m[:])
            xm = ym
            for j in range(8):
                nc.scalar.activation(xm[:, 512 * j:512 * (j + 1)],
                                     ym[:, 512 * j:512 * (j + 1)], AF.Relu,
                                     bias=b1m[:, j:j + 1], scale=s1m[:, j:j + 1])

            # ============ merge layer 2 + global BN ============
            pm2 = pp.tile([128, 512], F32, name="ps", tag="ps")
            for k in range(8):
                nc.tensor.matmul(pm2[:], wm2[:, 128 * k:128 * (k + 1)],
                                 xm[:, 512 * k:512 * (k + 1)],
                                 start=(k == 0), stop=(k == 7))
            ym2 = wp.tile([128, 512], F32, name="ym2")
            nc.scalar.copy(ym2[:], pm2[:])
            mst2 = wp.tile([128, 6], F32, name="mst2")
            nc.vector.bn_stats(mst2[:], ym2[:])
            b2i = dp.tile([128, 6], F32, name="b2i")
            nc.sync.dma_start(b2i[:], mst2[:])
            b2o = dp.tile([1024, 6], F32, name="b2o")
            nc.gpsimd.collective_compute("AllGather", OP.bypass, replica_groups=RG,
                                         ins=[b2i.opt()], outs=[b2o.opt()])
            keep_warm(22)
            gst2 = wp.tile([128, 48], F32, name="gst2")
            nc.sync.dma_start(gst2[:].rearrange("p (r s) -> p r s", r=8),
                              b2o[:].rearrange("(r p) s -> p r s", p=128))
            mv2 = wp.tile([128, 2], F32, name="mv2")
            nc.vector.bn_aggr(mv2[:], gst2[:].rearrange("p (r t e) -> p r t e",
                                                        r=8, e=3))
            rstd2m = _nr_rsqrt(nc, wp, mv2[:, 1:2], 1.0, "rm2")
            s2m = wp.tile([128, 1], F32, name="s2m")
            nc.vector.tensor_mul(s2m[:], rstd2m[:], gm2S[:])
            t2m = wp.tile([128, 1], F32, name="t2m")
            nc.vector.tensor_mul(t2m[:], mv2[:, 0:1], s2m[:])
            b2m = wp.tile([128, 1], F32, name="b2m")
            nc.vector.tensor_sub(b2m[:], bm2S[:], t2m[:])
            nc.scalar.activation(hT[:], ym2[:], AF.Relu,
                                 bias=b2m[:, 0:1], scale=s2m[:, 0:1])

        nc.sync.dma_start(d["h_out"].ap(), hT[:])


def _build():
    key = (T_STEPS, SIG_MODE)
    if key in _CACHE:
        return _CACHE[key]
    nc = bacc.Bacc("TRN2", target_bir_lowering=False, debug=False,
                   enable_asserts=True, num_devices=NCORES)
    d = {}
    ins = [
        ("hT", (128, 512)), ("cT", (128, 512)), ("posT", (2, 512)),
        ("relposT", (2, 512)),
        ("Wih", (64, 512)), ("Whh", (128, 512)), ("Wse", (2, 64)),
        ("Whp", (128, 2)), ("Wpse", (2, 64)), ("Wp1a", (64, 512)),
        ("Wp1b", (128, 512)), ("Wp2r", (128, 512)), ("Wm1r", (128, 2048)),
        ("Wm2r", (128, 1024)),
        ("bgS", (128, 4)), ("bgnS", (128, 4)), ("bg2S", (128, 4)),
        ("bseS", (64, 1)), ("bhpS", (2, 1)),
        ("gp1S", (128, 4)), ("bp1S", (128, 4)), ("gp2S", (128, 1)),
        ("bp2S", (128, 1)), ("gm1S", (128, 8)), ("bm1S", (128, 8)),
        ("gm2S", (128, 1)), ("bm2S", (128, 1)),
    ]
    for name, shape in ins:
        d[name] = nc.dram_tensor(name, list(shape), F32, kind="ExternalInput")
    d["traj"] = nc.dram_tensor("traj", [T_STEPS, 2, 512], F32, kind="ExternalOutput")
    d["h_out"] = nc.dram_tensor("h_out", [128, 512], F32, kind="ExternalOutput")
    with tile.TileContext(nc) as tc:
        _emit(nc, tc, d)
    nc.compile()
    _CACHE[key] = nc
    return nc


def _prep_inputs(inputs):
    f = lambda x: np.ascontiguousarray(np.asarray(x), dtype=np.float32)
    W_se, b_se = f(inputs["W_se"]), f(inputs["b_se"])
    W_hp, b_hp = f(inputs["W_hp"]), f(inputs["b_hp"])
    Wih, Whh = f(inputs["Wih"]), f(inputs["Whh"])
    bg = f(inputs["bih"]) + f(inputs["bhh"])
    Wp_se = f(inputs["Wp_se"])
    Wp1, Wp2 = f(inputs["Wp1"]), f(inputs["Wp2"])
    Wm1, Wm2 = f(inputs["Wm1"]), f(inputs["Wm2"])
    h0, c0 = f(inputs["h0"]), f(inputs["c0"])
    last_pos, last_pos_rel = f(inputs["last_pos"]), f(inputs["last_pos_rel"])

    col = lambda x: np.ascontiguousarray(x.reshape(-1, 1), np.float32)
    chunks = lambda x, n: np.ascontiguousarray(x.reshape(n, 128).T, np.float32)

    shared = {
        "Wih": Wih, "Whh": Whh, "Wse": W_se, "Whp": W_hp, "Wpse": Wp_se,
        "Wp1a": np.ascontiguousarray(Wp1[:64]),
        "Wp1b": np.ascontiguousarray(Wp1[64:]),
        "Wp2r": np.ascontiguousarray(
            Wp2.reshape(4, 128, 128).transpose(1, 0, 2).reshape(128, 512)),
        "Wm1r": np.ascontiguousarray(
            np.concatenate([Wm1[:128], Wm1[128:]], axis=1)),
        "Wm2r": np.ascontiguousarray(
            Wm2.reshape(8, 128, 128).transpose(1, 0, 2).reshape(128, 1024)),
        "bgS": chunks(bg, 4), "bgnS": chunks(-bg, 4), "bg2S": chunks(2 * bg, 4),
        "bseS": col(b_se), "bhpS": col(b_hp),
        "gp1S": chunks(f(inputs["gp1"]), 4), "bp1S": chunks(f(inputs["betap1"]), 4),
        "gp2S": col(f(inputs["gp2"])), "bp2S": col(f(inputs["betap2"])),
        "gm1S": chunks(f(inputs["gm1"]), 8), "bm1S": chunks(f(inputs["betam1"]), 8),
        "gm2S": col(f(inputs["gm2"])), "bm2S": col(f(inputs["betam2"])),
    }
    in_maps = []
    for i in range(NCORES):
        sl = slice(R * i, R * (i + 1))
        m = dict(shared)
        m["hT"] = np.ascontiguousarray(h0[0, sl].T)
        m["cT"] = np.ascontiguousarray(c0[0, sl].T)
        m["posT"] = np.ascontiguousarray(last_pos[sl].T)
        m["relposT"] = np.ascontiguousarray(last_pos_rel[sl].T)
        in_maps.append(m)
    return in_maps


def run_compiled(inputs, trace=False, **kw):
    nc = _build()
    in_maps = _prep_inputs(inputs)
    res = bass_utils.run_bass_kernel_spmd(nc, in_maps,
                                          core_ids=list(range(NCORES)),
                                          trace=trace, **kw)
    traj = np.empty((T_STEPS, NCORES * R, 2), np.float32)
    h = np.empty((1, NCORES * R, 128), np.float32)
    for i in range(NCORES):
        traj[:, R * i:R * (i + 1), :] = res.results[i]["traj"].transpose(0, 2, 1)
        h[0, R * i:R * (i + 1), :] = res.results[i]["h_out"].T
    return (traj, h), res


def kernel(**inputs):
    out, _ = run_compiled(inputs, trace=False)
    return out


# revision 10
# speedup vs baseline: 1.0538x; 1.0538x over previous
"""Trainium2 Bass kernel for nn_Decoder_39591008535099 (social-GAN style decoder).

Strategy
--------
Data-parallel over pedestrian groups: 8 NeuronCores, each owns 32 groups
(512 pedestrians). All weights replicated. Everything is computed in a
"transposed" layout: features on SBUF partitions, pedestrians on the free
axis, so BatchNorm statistics are free-axis reductions and matmuls chain as
out = W.T @ actT without any transposes.

Pool-net algebra (exact):
  Y1[a,b] = emb(pos_b - pos_a) @ Wp1a + h_b @ Wp1b + const
          = u[b] - v[a] + const,   u = pe@Wp1a + h@Wp1b, v = pe@Wp1a
  Per-group BN1 over the P*P grid: mean/var separate into u/v moments
  (Var = Var_b(u) + Var_a(v)); biases/embedding constants cancel in the
  centering, so they are dropped exactly.  X2 = relu(s*(u_c[b] - v_c[a]) + beta).
  BN2-then-max over b commutes with max since gamma2 > 0 (monotone affine),
  so the P*P tensor is reduced by max over raw Y2 first.

Merge-MLP BatchNorm needs exact global batch stats: each core computes
bn_stats partials, one 6KB AllGather per layer merges them (bn_aggr), which
is bit-faithful Welford pooling.  (Local per-shard stats diverge
catastrophically: this model amplifies per-step error ~5e4x over 12 steps.)

All matmuls f32 (bf16 is numerically fatal here), rsqrt is Newton-refined.
"""
import os
import numpy as np

try:
    import concourse.bass as bass
except ImportError:  # pragma: no cover
    import sys
    sys.path.insert(0, "/opt/trn_rl_repo")
    import concourse.bass as bass
import concourse.bacc as bacc
import concourse.tile as tile
import concourse.mybir as mybir
import concourse.bass_utils as bass_utils

AF = mybir.ActivationFunctionType
OP = mybir.AluOpType
AX = mybir.AxisListType
F32 = mybir.dt.float32

NCORES = 8
T_STEPS = int(os.environ.get("KERNEL_T_STEPS", "12"))
R = 512          # pedestrians per core
GS = 32          # groups per core
P = 16           # peds per group
EPS = 1e-5
SIG_MODE = os.environ.get("KERNEL_SIG_MODE", "native")   # "native" | "exp"

_CACHE = {}


def _nr_rsqrt(nc, wp, vraw, scale, name, rstd_mode="sqrt"):
    """rstd = 1/sqrt(vraw*scale + EPS), Newton-refined to ~1ulp.
    vraw: (p, n) AP. Returns SBUF tile (p, n)."""
    pdim, n = vraw.shape[0], vraw.shape[1] if len(vraw.shape) == 2 else None
    assert n is not None
    t = wp.tile([pdim, n], F32, name=f"{name}_t")
    nc.vector.tensor_scalar(out=t[:], in0=vraw, scalar1=float(scale),
                            scalar2=float(EPS), op0=OP.mult, op1=OP.add)
    r0 = wp.tile([pdim, n], F32, name=f"{name}_r0")
    if rstd_mode == "expln":
        l = wp.tile([pdim, n], F32, name=f"{name}_l")
        nc.scalar.activation(l[:], t[:], AF.Ln)
        nc.scalar.activation(r0[:], l[:], AF.Exp, scale=-0.5)
    else:
        sq = wp.tile([pdim, n], F32, name=f"{name}_sq")
        nc.scalar.activation(sq[:], t[:], AF.Sqrt)
        scr = wp.tile([pdim, n], F32, name=f"{name}_scr")
        nc.vector.reciprocal_approx_accurate(r0[:], sq[:], scr[:])
    # two Newton steps for rsqrt: r <- r*(1.5 - 0.5*t*r^2)
    r = r0
    for it in range(1):
        r2 = wp.tile([pdim, n], F32, name=f"{name}_r2{it}")
        nc.vector.tensor_mul(r2[:], r[:], r[:])
        tr = wp.tile([pdim, n], F32, name=f"{name}_tr{it}")
        nc.vector.tensor_mul(tr[:], t[:], r2[:])
        w = wp.tile([pdim, n], F32, name=f"{name}_w{it}")
        nc.vector.tensor_scalar(out=w[:], in0=tr[:], scalar1=-0.5, scalar2=1.5,
                                op0=OP.mult, op1=OP.add)
        rn = wp.tile([pdim, n], F32, name=f"{name}_rn{it}")
        nc.vector.tensor_mul(rn[:], r[:], w[:])
        r = rn
    return r


def _emit(nc, tc, d):
    RG = [list(range(NCORES))]
    with tc.tile_pool(name="const", bufs=1) as cp, \
         tc.tile_pool(name="work", bufs=2) as wp, \
         tc.tile_pool(name="psum", bufs=4, space="PSUM") as pp, \
         tc.tile_pool(name="dram", bufs=2, space="DRAM") as dp:

        def load(name, shape):
            tl = cp.tile(list(shape), F32, name=f"w_{name}")
            nc.sync.dma_start(tl[:], d[name].ap())
            return tl

        wih = load("Wih", (64, 512))
        whh = load("Whh", (128, 512))
        wse = load("Wse", (2, 64))
        whp = load("Whp", (128, 2))
        wpse = load("Wpse", (2, 64))
        wp1a = load("Wp1a", (64, 512))
        wp1b = load("Wp1b", (128, 512))
        wp2 = load("Wp2r", (128, 512))
        wm1 = load("Wm1r", (128, 2048))
        wm2 = load("Wm2r", (128, 1024))
        bgS = load("bgS", (128, 4))
        bgnS = load("bgnS", (128, 4))
        bg2S = load("bg2S", (128, 4))
        bseS = load("bseS", (64, 1))
        bhpS = load("bhpS", (2, 1))
        gp1S = load("gp1S", (128, 4))
        bp1S = load("bp1S", (128, 4))
        gp2S = load("gp2S", (128, 1))
        bp2S = load("bp2S", (128, 1))
        gm1S = load("gm1S", (128, 8))
        bm1S = load("bm1S", (128, 8))
        gm2S = load("gm2S", (128, 1))
        bm2S = load("bm2S", (128, 1))

        hT = load("hT", (128, 512))
        cT = load("cT", (128, 512))
        posT = load("posT", (2, 512))
        relpT = load("relposT", (2, 512))

        # initial dec_in = relpos @ Wse + bse
        decT = cp.tile([64, 512], F32, name="decT")
        pd0 = pp.tile([128, 512], F32, name="ps", tag="ps")
        nc.tensor.matmul(pd0[0:64, :], wse[:], relpT[:], start=True, stop=True)
        nc.vector.tensor_scalar_add(decT[:], pd0[0:64, :], bseS[:, 0:1])

        def act_sigmoid(dst, src, bias_col, nbias_col):
            if SIG_MODE == "native":
                nc.scalar.activation(dst[:], src, AF.Sigmoid, bias=bias_col, scale=1.0)
            else:
                e = wp.tile([128, 512], F32, name="sig_e")
                nc.scalar.activation(e[:], src, AF.Exp, bias=nbias_col, scale=-1.0)
                nc.vector.tensor_scalar_add(e[:], e[:], 1.0)
                scr = wp.tile([128, 512], F32, name="sig_scr")
                nc.vector.reciprocal_approx_accurate(dst[:], e[:], scr[:])

        def act_tanh(dst, src, bias_col, bias2_col):
            if SIG_MODE == "native":
                if bias_col is None:
                    nc.scalar.activation(dst[:], src, AF.Tanh)
                else:
                    nc.scalar.activation(dst[:], src, AF.Tanh, bias=bias_col, scale=1.0)
            else:
                e = wp.tile([128, 512], F32, name="tanh_e")
                if bias2_col is None:
                    nc.scalar.activation(e[:], src, AF.Exp, scale=2.0)
                else:
                    nc.scalar.activation(e[:], src, AF.Exp, bias=bias2_col, scale=2.0)
                nc.vector.tensor_scalar_add(e[:], e[:], 1.0)
                scr = wp.tile([128, 512], F32, name="tanh_scr")
                rec = wp.tile([128, 512], F32, name="tanh_rec")
                nc.vector.reciprocal_approx_accurate(rec[:], e[:], scr[:])
                nc.vector.tensor_scalar(out=dst[:], in0=rec[:], scalar1=-2.0,
                                        scalar2=1.0, op0=OP.mult, op1=OP.add)

        warm_ctr = [0]

        def keep_warm(n):
            return

        warm_sink = cp.tile([128, 1], F32, name="warm_sink")

        for t in range(T_STEPS):
            # ============ LSTM cell ============
            gact = []
            for j in range(4):
                if t == 0:
                    pg = pp.tile([128, 512], F32, name="ps", tag="ps")
                    nc.tensor.matmul(pg[:], wih[:, 128 * j:128 * (j + 1)], decT[:],
                                     start=True, stop=False)
                else:
                    pg = pg_next[j]
                nc.tensor.matmul(pg[:], whh[:, 128 * j:128 * (j + 1)], hT[:],
                                 start=False, stop=True)
                g_t = wp.tile([128, 512], F32, name=f"gact{j}", bufs=1)
                if j == 2:
                    act_tanh(g_t, pg[:], bgS[:, j:j + 1], bg2S[:, j:j + 1])
                else:
                    act_sigmoid(g_t, pg[:], bgS[:, j:j + 1], bgnS[:, j:j + 1])
                gact.append(g_t)
            keep_warm(10)
            t1 = wp.tile([128, 512], F32, name="lstm_t1")
            nc.vector.tensor_mul(t1[:], gact[1][:], cT[:])
            t2 = wp.tile([128, 512], F32, name="lstm_t2")
            nc.vector.tensor_mul(t2[:], gact[0][:], gact[2][:])
            nc.vector.tensor_add(cT[:], t1[:], t2[:])
            tcn = wp.tile([128, 512], F32, name="lstm_tc")
            act_tanh(tcn, cT[:], None, None)
            hn = wp.tile([128, 512], F32, name="hn")
            nc.vector.tensor_mul(hn[:], gact[3][:], tcn[:])

            # ============ rel_pos / pos / traj / dec_in ============
            prp = pp.tile([128, 512], F32, name="ps", tag="ps")
            nc.tensor.matmul(prp[0:2, :], whp[:], hn[:], start=True, stop=True)
            relT = wp.tile([2, 512], F32, name="relT")
            nc.vector.tensor_scalar_add(relT[:], prp[0:2, :], bhpS[:, 0:1])
            nc.vector.tensor_add(posT[:], posT[:], relT[:])
            nc.sync.dma_start(d["traj"].ap()[t], relT[:])

            pdi = pp.tile([128, 512], F32, name="ps", tag="ps")
            nc.tensor.matmul(pdi[0:64, :], wse[:], relT[:], start=True, stop=True)
            nc.vector.tensor_scalar_add(decT[:], pdi[0:64, :], bseS[:, 0:1])
            if t < T_STEPS - 1:
                pg_next = [pp.tile([128, 512], F32, name=f"pgn{j}", tag=f"pgn{j}",
                                   bufs=1) for j in range(4)]
                for j in range(4):
                    nc.tensor.matmul(pg_next[j][:], wih[:, 128 * j:128 * (j + 1)],
                                     decT[:], start=True, stop=False)

            # ============ pool: pe + input means ============
            ppe = pp.tile([128, 512], F32, name="ps", tag="ps")
            nc.tensor.matmul(ppe[0:64, :], wpse[:], posT[:], start=True, stop=True)
            peT = wp.tile([64, 512], F32, name="peT")
            nc.scalar.copy(peT[:], ppe[0:64, :])

            pe_m = wp.tile([64, 32], F32, name="pe_m")
            nc.vector.reduce_sum(pe_m[:],
                                 peT[:].rearrange("p (g b) -> p g b", b=P), axis=AX.X)
            pe_ne = wp.tile([64, 512], F32, name="pe_ne")
            nc.vector.tensor_scalar_mul(
                pe_ne[:].rearrange("p (g b) -> p g b", b=P),
                pe_m[:].unsqueeze(2).broadcast_to((64, GS, P)), -1.0 / P)
            h_m = wp.tile([128, 32], F32, name="h_m")
            nc.vector.reduce_sum(h_m[:],
                                 hn[:].rearrange("p (g b) -> p g b", b=P), axis=AX.X)
            h_ne = wp.tile([128, 512], F32, name="h_ne")
            nc.vector.tensor_scalar_mul(
                h_ne[:].rearrange("p (g b) -> p g b", b=P),
                h_m[:].unsqueeze(2).broadcast_to((128, GS, P)), -1.0 / P)

            # ============ pool: centered v and u, variances ============
            vcS = wp.tile([128, 2048], F32, name="vcS", bufs=1)
            ucS = wp.tile([128, 2048], F32, name="ucS", bufs=1)
            sqv = wp.tile([128, 2048], F32, name="sqv", bufs=1)
            squ = wp.tile([128, 2048], F32, name="squ", bufs=1)
            for j in range(4):
                jsl = slice(128 * j, 128 * (j + 1))
                pv = pp.tile([128, 512], F32, name="ps", tag="ps")
                nc.tensor.matmul(pv[:], wp1a[:, jsl], peT[:], start=True, stop=False)
                nc.tensor.matmul(pv[:], wp1a[:, jsl], pe_ne[:], start=False, stop=True)
                nc.scalar.copy(vcS[:, 512 * j:512 * (j + 1)], pv[:])
                nc.scalar.square(sqv[:, 512 * j:512 * (j + 1)], pv[:])
            for j in range(4):
                jsl = slice(128 * j, 128 * (j + 1))
                pu = pp.tile([128, 512], F32, name="ps", tag="ps")
                nc.tensor.matmul(pu[:], wp1b[:, jsl], hn[:], start=True, stop=False)
                nc.tensor.matmul(pu[:], wp1b[:, jsl], h_ne[:], start=False, stop=False)
                nc.tensor.matmul(pu[:], wp1a[:, jsl], peT[:], start=False, stop=False)
                nc.tensor.matmul(pu[:], wp1a[:, jsl], pe_ne[:], start=False, stop=True)
                nc.scalar.copy(ucS[:, 512 * j:512 * (j + 1)], pu[:])
                nc.scalar.square(squ[:, 512 * j:512 * (j + 1)], pu[:])

            vss = wp.tile([128, 128], F32, name="vss")
            nc.vector.reduce_sum(
                vss[:].rearrange("p (c g) -> p c g", g=GS),
                sqv[:].rearrange("p (c g b) -> p c g b", g=GS, b=P), axis=AX.X)
            uss = wp.tile([128, 128], F32, name="uss")
            nc.vector.reduce_sum(
                uss[:].rearrange("p (c g) -> p c g", g=GS),
                squ[:].rearrange("p (c g b) -> p c g b", g=GS, b=P), axis=AX.X)

            vv = wp.tile([128, 128], F32, name="vv")
            nc.vector.tensor_add(vv[:], uss[:], vss[:])
            rstd1 = _nr_rsqrt(nc, wp, vv[:], 1.0 / P, "rs1")
            s1 = wp.tile([128, 128], F32, name="s1")
            for j in range(4):
                nc.vector.tensor_scalar_mul(s1[:, 32 * j:32 * (j + 1)],
                                            rstd1[:, 32 * j:32 * (j + 1)],
                                            gp1S[:, j:j + 1])
            s1e = wp.tile([128, 2048], F32, name="sqv", bufs=1)
            nc.vector.tensor_copy(
                s1e[:].rearrange("p (c g b) -> p c g b", g=GS, b=P),
                s1[:].rearrange("p (c g) -> p c g", g=GS)
                    .unsqueeze(3).broadcast_to((128, 4, GS, P)))
            ucp = ucS
            nc.vector.tensor_mul(ucp[:], ucS[:], s1e[:])
            for j in range(4):
                nc.vector.tensor_scalar_add(ucp[:, 512 * j:512 * (j + 1)],
                                            ucp[:, 512 * j:512 * (j + 1)],
                                            bp1S[:, j:j + 1])
            vcp = vcS
            nc.vector.tensor_mul(vcp[:], vcS[:], s1e[:])

            # ============ pool: per-quad X2 -> Y2 -> stats/max ============
            poolmax = wp.tile([128, 512], F32, name="poolmax", bufs=1)
            mstY = wp.tile([128, 192], F32, name="mstY", bufs=1)
            for q in range(8):
                x2t = []
                for j in range(4):
                    x2 = wp.tile([128, 1024], F32, name=f"x2_{j}", bufs=2)
                    base = 512 * j + 64 * q
                    u_ap = (ucp[:, base:base + 64]
                            .rearrange("p (g b) -> p g b", b=P)
                            .unsqueeze(2).broadcast_to((128, 4, P, P)))
                    v_ap = (vcp[:, base:base + 64]
                            .rearrange("p (g a) -> p g a", a=P)
                            .unsqueeze(3).broadcast_to((128, 4, P, P)))
                    if j == 3:
                        nc.gpsimd.tensor_sub(
                            x2[:].rearrange("p (g a b) -> p g a b", a=P, b=P),
                            u_ap, v_ap)
                    else:
                        nc.vector.tensor_sub(
                            x2[:].rearrange("p (g a b) -> p g a b", a=P, b=P),
                            u_ap, v_ap)
                    if j < 2:
                        nc.scalar.activation(x2[:], x2[:], AF.Relu)
                    else:
                        nc.vector.tensor_scalar_max(x2[:], x2[:], 0.0)
                    x2t.append(x2)
                for h2 in range(2):
                    py2 = pp.tile([128, 512], F32, name="ps", tag="ps")
                    for j in range(4):
                        nc.tensor.matmul(py2[:], wp2[:, 128 * j:128 * (j + 1)],
                                         x2t[j][:, 512 * h2:512 * (h2 + 1)],
                                         start=(j == 0), stop=(j == 3))
                    gbase = 4 * q + 2 * h2
                    for g2 in range(2):
                        nc.vector.bn_stats(
                            mstY[:, 6 * (gbase + g2):6 * (gbase + g2) + 6],
                            py2[:, 256 * g2:256 * (g2 + 1)])
                    nc.vector.reduce_max(
                        poolmax[:, 64 * q + 32 * h2:64 * q + 32 * (h2 + 1)]
                        .rearrange("p (g a) -> p g a", a=P),
                        py2[:].rearrange("p (g a b) -> p g a b", a=P, b=P),
                        axis=AX.X)

            # ============ pool BN2 (monotone max trick) ============
            mr = mstY[:].rearrange("p (g s) -> p g s", s=6)
            dm = wp.tile([128, 32], F32, name="dm")
            nc.vector.tensor_sub(dm[:], mr[:, :, 1], mr[:, :, 4])
            dm2 = wp.tile([128, 32], F32, name="dm2")
            nc.vector.tensor_mul(dm2[:], dm[:], dm[:])
            cv = wp.tile([128, 32], F32, name="cvt")
            nc.vector.tensor_add(cv[:], mr[:, :, 2], mr[:, :, 5])
            v256 = wp.tile([128, 32], F32, name="v256")
            nc.vector.scalar_tensor_tensor(out=v256[:], in0=dm2[:], scalar=64.0,
                                           in1=cv[:], op0=OP.mult, op1=OP.add)
            msum = wp.tile([128, 32], F32, name="msum")
            nc.vector.tensor_add(msum[:], mr[:, :, 1], mr[:, :, 4])
            rstd2 = _nr_rsqrt(nc, wp, v256[:], 1.0 / 256.0, "rs2")
            s2 = wp.tile([128, 32], F32, name="s2")
            nc.vector.tensor_scalar_mul(s2[:], rstd2[:], gp2S[:, 0:1])
            ms2 = wp.tile([128, 32], F32, name="ms2")
            nc.vector.tensor_mul(ms2[:], msum[:], s2[:])
            ms2b = wp.tile([128, 32], F32, name="ms2b")
            nc.vector.tensor_scalar(out=ms2b[:], in0=ms2[:], scalar1=0.5,
                                    scalar2=bp2S[:, 0:1], op0=OP.mult,
                                    op1=OP.subtract)
            s2e = wp.tile([128, 512], F32, name="s2e", bufs=1)
            nc.vector.tensor_copy(
                s2e[:].rearrange("p (g b) -> p g b", b=P),
                s2[:].unsqueeze(2).broadcast_to((128, GS, P)))
            mbe = wp.tile([128, 512], F32, name="mbe", bufs=1)
            nc.vector.tensor_copy(
                mbe[:].rearrange("p (g b) -> p g b", b=P),
                ms2b[:].unsqueeze(2).broadcast_to((128, GS, P)))
            pms = wp.tile([128, 512], F32, name="pms", bufs=1)
            nc.vector.tensor_mul(pms[:], poolmax[:], s2e[:])
            poolT = wp.tile([128, 512], F32, name="poolT", bufs=1)
            nc.vector.tensor_sub(poolT[:], pms[:], mbe[:])
            nc.scalar.activation(poolT[:], poolT[:], AF.Relu)

            # ============ merge layer 1 + global BN (AllGather) ============
            ym = wp.tile([128, 4096], F32, name="ym", bufs=1)
            mst1 = wp.tile([128, 48], F32, name="mst1")
            for j in range(8):
                pm1 = pp.tile([128, 512], F32, name="ps", tag="ps")
                nc.tensor.matmul(pm1[:], wm1[:, 128 * j:128 * (j + 1)], hn[:],
                                 start=True, stop=False)
                nc.tensor.matmul(pm1[:], wm1[:, 1024 + 128 * j:1024 + 128 * (j + 1)],
                                 poolT[:], start=False, stop=True)
                nc.scalar.copy(ym[:, 512 * j:512 * (j + 1)], pm1[:])
                nc.vector.bn_stats(mst1[:, 6 * j:6 * j + 6],
                                   ym[:, 512 * j:512 * (j + 1)])
            b1i = dp.tile([128, 48], F32, name="b1i")
            nc.sync.dma_start(b1i[:], mst1[:])
            b1o = dp.tile([1024, 48], F32, name="b1o")
            nc.gpsimd.collective_compute("AllGather", OP.bypass, replica_groups=RG,
                                         ins=[b1i.opt()], outs=[b1o.opt()])
            keep_warm(30)
            gst1 = wp.tile([128, 384], F32, name="gst1")
            nc.sync.dma_start(gst1[:].rearrange("p (r s) -> p r s", r=8),
                              b1o[:].rearrange("(r p) s -> p r s", p=128))
            mv1 = wp.tile([128, 16], F32, name="mv1")
            g1r = gst1[:].rearrange("p (r c t e) -> p r c t e", r=8, c=8, e=3)
            for j in range(8):
                nc.vector.bn_aggr(mv1[:, 2 * j:2 * j + 2], g1r[:, :, j])
            mv1r = mv1[:].rearrange("p (j k) -> p j k", k=2)
            rstd1m = _nr_rsqrt(nc, wp, mv1r[:, :, 1], 1.0, "rm1")
            s1m = wp.tile([128, 8], F32, name="s1m")
            nc.vector.tensor_mul(s1m[:], rstd1m[:], gm1S[:])
            t1m = wp.tile([128, 8], F32, name="t1m")
            nc.vector.tensor_mul(t1m[:], mv1r[:, :, 0], s1m[:])
            b1m = wp.tile([128, 8], F32, name="b1m")
            nc.vector.tensor_sub(b1m[:], bm1S[:], t1m[:])
            xm = ym
            for j in range(8):
                nc.scalar.activation(xm[:, 512 * j:512 * (j + 1)],
                                     ym[:, 512 * j:512 * (j + 1)], AF.Relu,
                                     bias=b1m[:, j:j + 1], scale=s1m[:, j:j + 1])

            # ============ merge layer 2 + global BN ============
            pm2a = pp.tile([128, 512], F32, name="ps", tag="ps")
            pm2b = pp.tile([128, 512], F32, name="ps", tag="ps")
            for k in range(4):
                nc.tensor.matmul(pm2a[:], wm2[:, 128 * k:128 * (k + 1)],
                                 xm[:, 512 * k:512 * (k + 1)],
                                 start=(k == 0), stop=(k == 3))
            for k in range(4, 8):
                nc.tensor.matmul(pm2b[:], wm2[:, 128 * k:128 * (k + 1)],
                                 xm[:, 512 * k:512 * (k + 1)],
                                 start=(k == 4), stop=(k == 7))
            ym2 = wp.tile([128, 512], F32, name="ym2")
            nc.scalar.copy(ym2[:], pm2a[:])
            nc.vector.tensor_add(ym2[:], ym2[:], pm2b[:])
            mst2 = wp.tile([128, 6], F32, name="mst2")
            nc.vector.bn_stats(mst2[:], ym2[:])
            b2i = dp.tile([128, 6], F32, name="b2i")
            nc.sync.dma_start(b2i[:], mst2[:])
            b2o = dp.tile([1024, 6], F32, name="b2o")
            nc.gpsimd.collective_compute("AllGather", OP.bypass, replica_groups=RG,
                                         ins=[b2i.opt()], outs=[b2o.opt()])
            keep_warm(22)
            gst2 = wp.tile([128, 48], F32, name="gst2")
            nc.sync.dma_start(gst2[:].rearrange("p (r s) -> p r s", r=8),
                              b2o[:].rearrange("(r p) s -> p r s", p=128))
            mv2 = wp.tile([128, 2], F32, name="mv2")
            nc.vector.bn_aggr(mv2[:], gst2[:].rearrange("p (r t e) -> p r t e",
                                                        r=8, e=3))
            rstd2m = _nr_rsqrt(nc, wp, mv2[:, 1:2], 1.0, "rm2")
            s2m = wp.tile([128, 1], F32, name="s2m")
            nc.vector.tensor_mul(s2m[:], rstd2m[:], gm2S[:])
            t2m = wp.tile([128, 1], F32, name="t2m")
            nc.vector.tensor_mul(t2m[:], mv2[:, 0:1], s2m[:])
            b2m = wp.tile([128, 1], F32, name="b2m")
            nc.vector.tensor_sub(b2m[:], bm2S[:], t2m[:])
            nc.scalar.activation(hT[:], ym2[:], AF.Relu,
                                 bias=b2m[:, 0:1], scale=s2m[:, 0:1])

        nc.sync.dma_start(d["h_out"].ap(), hT[:])


def _build():
    key = (T_STEPS, SIG_MODE)
    if key in _CACHE:
        return _CACHE[key]
    nc = bacc.Bacc("TRN2", target_bir_lowering=False, debug=False,
                   enable_asserts=True, num_devices=NCORES)
    d = {}
    ins = [
        ("hT", (128, 512)), ("cT", (128, 512)), ("posT", (2, 512)),
        ("relposT", (2, 512)),
        ("Wih", (64, 512)), ("Whh", (128, 512)), ("Wse", (2, 64)),
        ("Whp", (128, 2)), ("Wpse", (2, 64)), ("Wp1a", (64, 512)),
        ("Wp1b", (128, 512)), ("Wp2r", (128, 512)), ("Wm1r", (128, 2048)),
        ("Wm2r", (128, 1024)),
        ("bgS", (128, 4)), ("bgnS", (128, 4)), ("bg2S", (128, 4)),
        ("bseS", (64, 1)), ("bhpS", (2, 1)),
        ("gp1S", (128, 4)), ("bp1S", (128, 4)), ("gp2S", (128, 1)),
        ("bp2S", (128, 1)), ("gm1S", (128, 8)), ("bm1S", (128, 8)),
        ("gm2S", (128, 1)), ("bm2S", (128, 1)),
    ]
    for name, shape in ins:
        d[name] = nc.dram_tensor(name, list(shape), F32, kind="ExternalInput")
    d["traj"] = nc.dram_tensor("traj", [T_STEPS, 2, 512], F32, kind="ExternalOutput")
    d["h_out"] = nc.dram_tensor("h_out", [128, 512], F32, kind="ExternalOutput")
    with tile.TileContext(nc) as tc:
        _emit(nc, tc, d)
    nc.compile()
    _CACHE[key] = nc
    return nc


def _prep_inputs(inputs):
    f = lambda x: np.ascontiguousarray(np.asarray(x), dtype=np.float32)
    W_se, b_se = f(inputs["W_se"]), f(inputs["b_se"])
    W_hp, b_hp = f(inputs["W_hp"]), f(inputs["b_hp"])
    Wih, Whh = f(inputs["Wih"]), f(inputs["Whh"])
    bg = f(inputs["bih"]) + f(inputs["bhh"])
    Wp_se = f(inputs["Wp_se"])
    Wp1, Wp2 = f(inputs["Wp1"]), f(inputs["Wp2"])
    Wm1, Wm2 = f(inputs["Wm1"]), f(inputs["Wm2"])
    h0, c0 = f(inputs["h0"]), f(inputs["c0"])
    last_pos, last_pos_rel = f(inputs["last_pos"]), f(inputs["last_pos_rel"])

    col = lambda x: np.ascontiguousarray(x.reshape(-1, 1), np.float32)
    chunks = lambda x, n: np.ascontiguousarray(x.reshape(n, 128).T, np.float32)

    shared = {
        "Wih": Wih, "Whh": Whh, "Wse": W_se, "Whp": W_hp, "Wpse": Wp_se,
        "Wp1a": np.ascontiguousarray(Wp1[:64]),
        "Wp1b": np.ascontiguousarray(Wp1[64:]),
        "Wp2r": np.ascontiguousarray(
            Wp2.reshape(4, 128, 128).transpose(1, 0, 2).reshape(128, 512)),
        "Wm1r": np.ascontiguousarray(
            np.concatenate([Wm1[:128], Wm1[128:]], axis=1)),
        "Wm2r": np.ascontiguousarray(
            Wm2.reshape(8, 128, 128).transpose(1, 0, 2).reshape(128, 1024)),
        "bgS": chunks(bg, 4), "bgnS": chunks(-bg, 4), "bg2S": chunks(2 * bg, 4),
        "bseS": col(b_se), "bhpS": col(b_hp),
        "gp1S": chunks(f(inputs["gp1"]), 4), "bp1S": chunks(f(inputs["betap1"]), 4),
        "gp2S": col(f(inputs["gp2"])), "bp2S": col(f(inputs["betap2"])),
        "gm1S": chunks(f(inputs["gm1"]), 8), "bm1S": chunks(f(inputs["betam1"]), 8),
        "gm2S": col(f(inputs["gm2"])), "bm2S": col(f(inputs["betam2"])),
    }
    in_maps = []
    for i in range(NCORES):
        sl = slice(R * i, R * (i + 1))
        m = dict(shared)
        m["hT"] = np.ascontiguousarray(h0[0, sl].T)
        m["cT"] = np.ascontiguousarray(c0[0, sl].T)
        m["posT"] = np.ascontiguousarray(last_pos[sl].T)
        m["relposT"] = np.ascontiguousarray(last_pos_rel[sl].T)
        in_maps.append(m)
    return in_maps


def run_compiled(inputs, trace=False, **kw):
    nc = _build()
    in_maps = _prep_inputs(inputs)
    res = bass_utils.run_bass_kernel_spmd(nc, in_maps,
                                          core_ids=list(range(NCORES)),
                                          trace=trace, **kw)
    traj = np.empty((T_STEPS, NCORES * R, 2), np.float32)
    h = np.empty((1, NCORES * R, 128), np.float32)
    for i in range(NCORES):
        traj[:, R * i:R * (i + 1), :] = res.results[i]["traj"].transpose(0, 2, 1)
        h[0, R * i:R * (i + 1), :] = res.results[i]["h_out"].T
    return (traj, h), res


def kernel(**inputs):
    out, _ = run_compiled(inputs, trace=False)
    return out


# revision 11
# speedup vs baseline: 1.1141x; 1.0572x over previous
"""Trainium2 Bass kernel for nn_Decoder_39591008535099 (social-GAN style decoder).

Strategy
--------
Data-parallel over pedestrian groups: 8 NeuronCores, each owns 32 groups
(512 pedestrians). All weights replicated. Everything is computed in a
"transposed" layout: features on SBUF partitions, pedestrians on the free
axis, so BatchNorm statistics are free-axis reductions and matmuls chain as
out = W.T @ actT without any transposes.

Pool-net algebra (exact):
  Y1[a,b] = emb(pos_b - pos_a) @ Wp1a + h_b @ Wp1b + const
          = u[b] - v[a] + const,   u = pe@Wp1a + h@Wp1b, v = pe@Wp1a
  Per-group BN1 over the P*P grid: mean/var separate into u/v moments
  (Var = Var_b(u) + Var_a(v)); biases/embedding constants cancel in the
  centering, so they are dropped exactly.  X2 = relu(s*(u_c[b] - v_c[a]) + beta).
  BN2-then-max over b commutes with max since gamma2 > 0 (monotone affine),
  so the P*P tensor is reduced by max over raw Y2 first.

Merge-MLP BatchNorm needs exact global batch stats: each core computes
bn_stats partials, one 6KB AllGather per layer merges them (bn_aggr), which
is bit-faithful Welford pooling.  (Local per-shard stats diverge
catastrophically: this model amplifies per-step error ~5e4x over 12 steps.)

All matmuls f32 (bf16 is numerically fatal here), rsqrt is Newton-refined.
"""
import os
import numpy as np

try:
    import concourse.bass as bass
except ImportError:  # pragma: no cover
    import sys
    sys.path.insert(0, "/opt/trn_rl_repo")
    import concourse.bass as bass
import concourse.bacc as bacc
import concourse.tile as tile
import concourse.mybir as mybir
import concourse.bass_utils as bass_utils

AF = mybir.ActivationFunctionType
OP = mybir.AluOpType
AX = mybir.AxisListType
F32 = mybir.dt.float32

NCORES = 8
T_STEPS = int(os.environ.get("KERNEL_T_STEPS", "12"))
R = 512          # pedestrians per core
GS = 32          # groups per core
P = 16           # peds per group
EPS = 1e-5
SIG_MODE = os.environ.get("KERNEL_SIG_MODE", "native")   # "native" | "exp"

_CACHE = {}


def _nr_rsqrt(nc, wp, vraw, scale, name, rstd_mode="sqrt"):
    """rstd = 1/sqrt(vraw*scale + EPS), Newton-refined to ~1ulp.
    vraw: (p, n) AP. Returns SBUF tile (p, n)."""
    pdim, n = vraw.shape[0], vraw.shape[1] if len(vraw.shape) == 2 else None
    assert n is not None
    t = wp.tile([pdim, n], F32, name=f"{name}_t")
    nc.vector.tensor_scalar(out=t[:], in0=vraw, scalar1=float(scale),
                            scalar2=float(EPS), op0=OP.mult, op1=OP.add)
    r0 = wp.tile([pdim, n], F32, name=f"{name}_r0")
    if rstd_mode == "expln":
        l = wp.tile([pdim, n], F32, name=f"{name}_l")
        nc.scalar.activation(l[:], t[:], AF.Ln)
        nc.scalar.activation(r0[:], l[:], AF.Exp, scale=-0.5)
    else:
        sq = wp.tile([pdim, n], F32, name=f"{name}_sq")
        nc.scalar.activation(sq[:], t[:], AF.Sqrt)
        scr = wp.tile([pdim, n], F32, name=f"{name}_scr")
        nc.vector.reciprocal_approx_accurate(r0[:], sq[:], scr[:])
    # two Newton steps for rsqrt: r <- r*(1.5 - 0.5*t*r^2)
    r = r0
    for it in range(1):
        r2 = wp.tile([pdim, n], F32, name=f"{name}_r2{it}")
        nc.vector.tensor_mul(r2[:], r[:], r[:])
        tr = wp.tile([pdim, n], F32, name=f"{name}_tr{it}")
        nc.vector.tensor_mul(tr[:], t[:], r2[:])
        w = wp.tile([pdim, n], F32, name=f"{name}_w{it}")
        nc.vector.tensor_scalar(out=w[:], in0=tr[:], scalar1=-0.5, scalar2=1.5,
                                op0=OP.mult, op1=OP.add)
        rn = wp.tile([pdim, n], F32, name=f"{name}_rn{it}")
        nc.vector.tensor_mul(rn[:], r[:], w[:])
        r = rn
    return r


def _emit(nc, tc, d):
    RG = [list(range(NCORES))]
    with tc.tile_pool(name="const", bufs=1) as cp, \
         tc.tile_pool(name="work", bufs=2) as wp, \
         tc.tile_pool(name="psum", bufs=4, space="PSUM") as pp, \
         tc.tile_pool(name="dram", bufs=2, space="DRAM") as dp:

        def load(name, shape):
            tl = cp.tile(list(shape), F32, name=f"w_{name}")
            nc.sync.dma_start(tl[:], d[name].ap())
            return tl

        wih = load("Wih", (64, 512))
        whh = load("Whh", (128, 512))
        wse = load("Wse", (2, 64))
        whp = load("Whp", (128, 2))
        wpse = load("Wpse", (2, 64))
        wp1a = load("Wp1a", (64, 512))
        wp1b = load("Wp1b", (128, 512))
        wp2 = load("Wp2r", (128, 512))
        wm1 = load("Wm1r", (128, 2048))
        wm2 = load("Wm2r", (128, 1024))
        bgS = load("bgS", (128, 4))
        bgnS = load("bgnS", (128, 4))
        bg2S = load("bg2S", (128, 4))
        bseS = load("bseS", (64, 1))
        bhpS = load("bhpS", (2, 1))
        gp1S = load("gp1S", (128, 4))
        bp1S = load("bp1S", (128, 4))
        gp2S = load("gp2S", (128, 1))
        bp2S = load("bp2S", (128, 1))
        gm1S = load("gm1S", (128, 8))
        bm1S = load("bm1S", (128, 8))
        gm2S = load("gm2S", (128, 1))
        bm2S = load("bm2S", (128, 1))

        hT = load("hT", (128, 512))
        cT = load("cT", (128, 512))
        posT = load("posT", (2, 512))
        relpT = load("relposT", (2, 512))

        # initial dec_in = relpos @ Wse + bse
        decT = cp.tile([64, 512], F32, name="decT")
        pd0 = pp.tile([128, 512], F32, name="ps", tag="ps")
        nc.tensor.matmul(pd0[0:64, :], wse[:], relpT[:], start=True, stop=True)
        nc.vector.tensor_scalar_add(decT[:], pd0[0:64, :], bseS[:, 0:1])

        def act_sigmoid(dst, src, bias_col, nbias_col):
            if SIG_MODE == "native":
                nc.scalar.activation(dst[:], src, AF.Sigmoid, bias=bias_col, scale=1.0)
            else:
                e = wp.tile([128, 512], F32, name="sig_e")
                nc.scalar.activation(e[:], src, AF.Exp, bias=nbias_col, scale=-1.0)
                nc.vector.tensor_scalar_add(e[:], e[:], 1.0)
                scr = wp.tile([128, 512], F32, name="sig_scr")
                nc.vector.reciprocal_approx_accurate(dst[:], e[:], scr[:])

        def act_tanh(dst, src, bias_col, bias2_col):
            if SIG_MODE == "native":
                if bias_col is None:
                    nc.scalar.activation(dst[:], src, AF.Tanh)
                else:
                    nc.scalar.activation(dst[:], src, AF.Tanh, bias=bias_col, scale=1.0)
            else:
                e = wp.tile([128, 512], F32, name="tanh_e")
                if bias2_col is None:
                    nc.scalar.activation(e[:], src, AF.Exp, scale=2.0)
                else:
                    nc.scalar.activation(e[:], src, AF.Exp, bias=bias2_col, scale=2.0)
                nc.vector.tensor_scalar_add(e[:], e[:], 1.0)
                scr = wp.tile([128, 512], F32, name="tanh_scr")
                rec = wp.tile([128, 512], F32, name="tanh_rec")
                nc.vector.reciprocal_approx_accurate(rec[:], e[:], scr[:])
                nc.vector.tensor_scalar(out=dst[:], in0=rec[:], scalar1=-2.0,
                                        scalar2=1.0, op0=OP.mult, op1=OP.add)

        warm_ctr = [0]

        def keep_warm(n):
            return

        warm_sink = cp.tile([128, 1], F32, name="warm_sink")

        for t in range(T_STEPS):
            # ============ LSTM cell ============
            gact = []
            for j in range(4):
                if t == 0:
                    pg = pp.tile([128, 512], F32, name="ps", tag="ps")
                    nc.tensor.matmul(pg[:], wih[:, 128 * j:128 * (j + 1)], decT[:],
                                     start=True, stop=False)
                else:
                    pg = pg_next[j]
                nc.tensor.matmul(pg[:], whh[:, 128 * j:128 * (j + 1)], hT[:],
                                 start=False, stop=True)
                g_t = wp.tile([128, 512], F32, name=f"gact{j}", bufs=1)
                if j == 2:
                    act_tanh(g_t, pg[:], bgS[:, j:j + 1], bg2S[:, j:j + 1])
                else:
                    act_sigmoid(g_t, pg[:], bgS[:, j:j + 1], bgnS[:, j:j + 1])
                gact.append(g_t)
            keep_warm(10)
            t1 = wp.tile([128, 512], F32, name="lstm_t1")
            nc.vector.tensor_mul(t1[:], gact[1][:], cT[:])
            t2 = wp.tile([128, 512], F32, name="lstm_t2")
            nc.vector.tensor_mul(t2[:], gact[0][:], gact[2][:])
            nc.vector.tensor_add(cT[:], t1[:], t2[:])
            tcn = wp.tile([128, 512], F32, name="lstm_tc")
            act_tanh(tcn, cT[:], None, None)
            hn = wp.tile([128, 512], F32, name="hn")
            nc.vector.tensor_mul(hn[:], gact[3][:], tcn[:])

            # ============ rel_pos / pos / traj / dec_in ============
            prp = pp.tile([128, 512], F32, name="ps", tag="ps")
            nc.tensor.matmul(prp[0:2, :], whp[:], hn[:], start=True, stop=True)
            relT = wp.tile([2, 512], F32, name="relT")
            nc.vector.tensor_scalar_add(relT[:], prp[0:2, :], bhpS[:, 0:1])
            nc.vector.tensor_add(posT[:], posT[:], relT[:])
            nc.sync.dma_start(d["traj"].ap()[t], relT[:])

            pdi = pp.tile([128, 512], F32, name="ps", tag="ps")
            nc.tensor.matmul(pdi[0:64, :], wse[:], relT[:], start=True, stop=True)
            nc.vector.tensor_scalar_add(decT[:], pdi[0:64, :], bseS[:, 0:1])
            if t < T_STEPS - 1:
                pg_next = [pp.tile([128, 512], F32, name=f"pgn{j}", tag=f"pgn{j}",
                                   bufs=1) for j in range(4)]
                for j in range(4):
                    nc.tensor.matmul(pg_next[j][:], wih[:, 128 * j:128 * (j + 1)],
                                     decT[:], start=True, stop=False)

            # ============ pool: pe + input means ============
            ppe = pp.tile([128, 512], F32, name="ps", tag="ps")
            nc.tensor.matmul(ppe[0:64, :], wpse[:], posT[:], start=True, stop=True)
            peT = wp.tile([64, 512], F32, name="peT")
            nc.scalar.copy(peT[:], ppe[0:64, :])

            pe_m = wp.tile([64, 32], F32, name="pe_m")
            nc.vector.reduce_sum(pe_m[:],
                                 peT[:].rearrange("p (g b) -> p g b", b=P), axis=AX.X)
            pe_ne = wp.tile([64, 512], F32, name="pe_ne")
            nc.vector.tensor_scalar_mul(
                pe_ne[:].rearrange("p (g b) -> p g b", b=P),
                pe_m[:].unsqueeze(2).broadcast_to((64, GS, P)), -1.0 / P)
            h_m = wp.tile([128, 32], F32, name="h_m")
            nc.vector.reduce_sum(h_m[:],
                                 hn[:].rearrange("p (g b) -> p g b", b=P), axis=AX.X)
            h_ne = wp.tile([128, 512], F32, name="h_ne")
            nc.vector.tensor_scalar_mul(
                h_ne[:].rearrange("p (g b) -> p g b", b=P),
                h_m[:].unsqueeze(2).broadcast_to((128, GS, P)), -1.0 / P)

            # ============ pool: centered v and u, variances ============
            vcS = wp.tile([128, 2048], F32, name="vcS", bufs=1)
            ucS = wp.tile([128, 2048], F32, name="ucS", bufs=1)
            sqv = wp.tile([128, 2048], F32, name="sqv", bufs=1)
            squ = wp.tile([128, 2048], F32, name="squ", bufs=1)
            vss = wp.tile([128, 128], F32, name="vss")
            uss = wp.tile([128, 128], F32, name="uss")
            s1 = wp.tile([128, 128], F32, name="s1")
            s1e = wp.tile([128, 2048], F32, name="s1e", bufs=1)
            ucp = ucS
            vcp = vcS
            for j in range(4):
                jsl = slice(128 * j, 128 * (j + 1))
                csl = slice(512 * j, 512 * (j + 1))
                gsl = slice(32 * j, 32 * (j + 1))
                pv = pp.tile([128, 512], F32, name="ps", tag="ps")
                nc.tensor.matmul(pv[:], wp1a[:, jsl], peT[:], start=True, stop=False)
                nc.tensor.matmul(pv[:], wp1a[:, jsl], pe_ne[:], start=False, stop=True)
                nc.scalar.copy(vcS[:, csl], pv[:])
                nc.scalar.square(sqv[:, csl], pv[:])
                pu = pp.tile([128, 512], F32, name="ps", tag="ps")
                nc.tensor.matmul(pu[:], wp1b[:, jsl], hn[:], start=True, stop=False)
                nc.tensor.matmul(pu[:], wp1b[:, jsl], h_ne[:], start=False, stop=False)
                nc.tensor.matmul(pu[:], wp1a[:, jsl], peT[:], start=False, stop=False)
                nc.tensor.matmul(pu[:], wp1a[:, jsl], pe_ne[:], start=False, stop=True)
                nc.scalar.copy(ucS[:, csl], pu[:])
                nc.scalar.square(squ[:, csl], pu[:])
                # per-chunk stats -> scale -> normalized u', v' (overlaps next chunk MMs)
                nc.vector.reduce_sum(
                    vss[:, gsl].rearrange("p (o g) -> p o g", o=1),
                    sqv[:, csl].rearrange("p (o g b) -> p o g b", o=1, g=GS, b=P),
                    axis=AX.X)
                nc.vector.reduce_sum(
                    uss[:, gsl].rearrange("p (o g) -> p o g", o=1),
                    squ[:, csl].rearrange("p (o g b) -> p o g b", o=1, g=GS, b=P),
                    axis=AX.X)
                vvj = wp.tile([128, 32], F32, name="vvj")
                nc.vector.tensor_add(vvj[:], uss[:, gsl], vss[:, gsl])
                rstd1j = _nr_rsqrt(nc, wp, vvj[:], 1.0 / P, "rs1")
                nc.vector.tensor_scalar_mul(s1[:, gsl], rstd1j[:], gp1S[:, j:j + 1])
                nc.vector.tensor_copy(
                    s1e[:, csl].rearrange("p (g b) -> p g b", b=P),
                    s1[:, gsl].unsqueeze(2).broadcast_to((128, GS, P)))
                nc.vector.tensor_mul(ucp[:, csl], ucS[:, csl], s1e[:, csl])
                nc.vector.tensor_scalar_add(ucp[:, csl], ucp[:, csl],
                                            bp1S[:, j:j + 1])
                nc.vector.tensor_mul(vcp[:, csl], vcS[:, csl], s1e[:, csl])

            # ============ pool: per-quad X2 -> Y2 -> stats/max ============
            poolmax = wp.tile([128, 512], F32, name="poolmax", bufs=1)
            mstY = wp.tile([128, 192], F32, name="mstY", bufs=1)
            for q in range(8):
                x2t = []
                for j in range(4):
                    x2 = wp.tile([128, 1024], F32, name=f"x2_{j}", bufs=2)
                    base = 512 * j + 64 * q
                    u_ap = (ucp[:, base:base + 64]
                            .rearrange("p (g b) -> p g b", b=P)
                            .unsqueeze(2).broadcast_to((128, 4, P, P)))
                    v_ap = (vcp[:, base:base + 64]
                            .rearrange("p (g a) -> p g a", a=P)
                            .unsqueeze(3).broadcast_to((128, 4, P, P)))
                    if j == 3:
                        nc.gpsimd.tensor_sub(
                            x2[:].rearrange("p (g a b) -> p g a b", a=P, b=P),
                            u_ap, v_ap)
                    else:
                        nc.vector.tensor_sub(
                            x2[:].rearrange("p (g a b) -> p g a b", a=P, b=P),
                            u_ap, v_ap)
                    if j < 2:
                        nc.scalar.activation(x2[:], x2[:], AF.Relu)
                    else:
                        nc.vector.tensor_scalar_max(x2[:], x2[:], 0.0)
                    x2t.append(x2)
                for h2 in range(2):
                    py2 = pp.tile([128, 512], F32, name="ps", tag="ps")
                    for j in range(4):
                        nc.tensor.matmul(py2[:], wp2[:, 128 * j:128 * (j + 1)],
                                         x2t[j][:, 512 * h2:512 * (h2 + 1)],
                                         start=(j == 0), stop=(j == 3))
                    gbase = 4 * q + 2 * h2
                    for g2 in range(2):
                        nc.vector.bn_stats(
                            mstY[:, 6 * (gbase + g2):6 * (gbase + g2) + 6],
                            py2[:, 256 * g2:256 * (g2 + 1)])
                    nc.vector.reduce_max(
                        poolmax[:, 64 * q + 32 * h2:64 * q + 32 * (h2 + 1)]
                        .rearrange("p (g a) -> p g a", a=P),
                        py2[:].rearrange("p (g a b) -> p g a b", a=P, b=P),
                        axis=AX.X)

            # ============ pool BN2 (monotone max trick) ============
            mr = mstY[:].rearrange("p (g s) -> p g s", s=6)
            dm = wp.tile([128, 32], F32, name="dm")
            nc.vector.tensor_sub(dm[:], mr[:, :, 1], mr[:, :, 4])
            dm2 = wp.tile([128, 32], F32, name="dm2")
            nc.vector.tensor_mul(dm2[:], dm[:], dm[:])
            cv = wp.tile([128, 32], F32, name="cvt")
            nc.vector.tensor_add(cv[:], mr[:, :, 2], mr[:, :, 5])
            v256 = wp.tile([128, 32], F32, name="v256")
            nc.vector.scalar_tensor_tensor(out=v256[:], in0=dm2[:], scalar=64.0,
                                           in1=cv[:], op0=OP.mult, op1=OP.add)
            msum = wp.tile([128, 32], F32, name="msum")
            nc.vector.tensor_add(msum[:], mr[:, :, 1], mr[:, :, 4])
            rstd2 = _nr_rsqrt(nc, wp, v256[:], 1.0 / 256.0, "rs2")
            s2 = wp.tile([128, 32], F32, name="s2")
            nc.vector.tensor_scalar_mul(s2[:], rstd2[:], gp2S[:, 0:1])
            ms2 = wp.tile([128, 32], F32, name="ms2")
            nc.vector.tensor_mul(ms2[:], msum[:], s2[:])
            ms2b = wp.tile([128, 32], F32, name="ms2b")
            nc.vector.tensor_scalar(out=ms2b[:], in0=ms2[:], scalar1=0.5,
                                    scalar2=bp2S[:, 0:1], op0=OP.mult,
                                    op1=OP.subtract)
            s2e = wp.tile([128, 512], F32, name="s2e", bufs=1)
            nc.vector.tensor_copy(
                s2e[:].rearrange("p (g b) -> p g b", b=P),
                s2[:].unsqueeze(2).broadcast_to((128, GS, P)))
            mbe = wp.tile([128, 512], F32, name="mbe", bufs=1)
            nc.vector.tensor_copy(
                mbe[:].rearrange("p (g b) -> p g b", b=P),
                ms2b[:].unsqueeze(2).broadcast_to((128, GS, P)))
            pms = wp.tile([128, 512], F32, name="pms", bufs=1)
            nc.vector.tensor_mul(pms[:], poolmax[:], s2e[:])
            poolT = wp.tile([128, 512], F32, name="poolT", bufs=1)
            nc.vector.tensor_sub(poolT[:], pms[:], mbe[:])
            nc.scalar.activation(poolT[:], poolT[:], AF.Relu)

            # ============ merge layer 1 + global BN (AllGather) ============
            ym = wp.tile([128, 4096], F32, name="ym", bufs=1)
            mst1 = wp.tile([128, 48], F32, name="mst1")
            for j in range(8):
                pm1 = pp.tile([128, 512], F32, name="ps", tag="ps")
                nc.tensor.matmul(pm1[:], wm1[:, 128 * j:128 * (j + 1)], hn[:],
                                 start=True, stop=False)
                nc.tensor.matmul(pm1[:], wm1[:, 1024 + 128 * j:1024 + 128 * (j + 1)],
                                 poolT[:], start=False, stop=True)
                nc.scalar.copy(ym[:, 512 * j:512 * (j + 1)], pm1[:])
                nc.vector.bn_stats(mst1[:, 6 * j:6 * j + 6],
                                   ym[:, 512 * j:512 * (j + 1)])
            b1i = dp.tile([128, 48], F32, name="b1i")
            nc.sync.dma_start(b1i[:], mst1[:])
            b1o = dp.tile([1024, 48], F32, name="b1o")
            nc.gpsimd.collective_compute("AllGather", OP.bypass, replica_groups=RG,
                                         ins=[b1i.opt()], outs=[b1o.opt()])
            keep_warm(30)
            gst1 = wp.tile([128, 384], F32, name="gst1")
            nc.sync.dma_start(gst1[:].rearrange("p (r s) -> p r s", r=8),
                              b1o[:].rearrange("(r p) s -> p r s", p=128))
            mv1 = wp.tile([128, 16], F32, name="mv1")
            g1r = gst1[:].rearrange("p (r c t e) -> p r c t e", r=8, c=8, e=3)
            for j in range(8):
                nc.vector.bn_aggr(mv1[:, 2 * j:2 * j + 2], g1r[:, :, j])
            mv1r = mv1[:].rearrange("p (j k) -> p j k", k=2)
            rstd1m = _nr_rsqrt(nc, wp, mv1r[:, :, 1], 1.0, "rm1")
            s1m = wp.tile([128, 8], F32, name="s1m")
            nc.vector.tensor_mul(s1m[:], rstd1m[:], gm1S[:])
            t1m = wp.tile([128, 8], F32, name="t1m")
            nc.vector.tensor_mul(t1m[:], mv1r[:, :, 0], s1m[:])
            b1m = wp.tile([128, 8], F32, name="b1m")
            nc.vector.tensor_sub(b1m[:], bm1S[:], t1m[:])
            xm = ym
            for j in range(8):
                nc.scalar.activation(xm[:, 512 * j:512 * (j + 1)],
                                     ym[:, 512 * j:512 * (j + 1)], AF.Relu,
                                     bias=b1m[:, j:j + 1], scale=s1m[:, j:j + 1])

            # ============ merge layer 2 + global BN ============
            pm2a = pp.tile([128, 512], F32, name="ps", tag="ps")
            pm2b = pp.tile([128, 512], F32, name="ps", tag="ps")
            for k in range(4):
                nc.tensor.matmul(pm2a[:], wm2[:, 128 * k:128 * (k + 1)],
                                 xm[:, 512 * k:512 * (k + 1)],
                                 start=(k == 0), stop=(k == 3))
            for k in range(4, 8):
                nc.tensor.matmul(pm2b[:], wm2[:, 128 * k:128 * (k + 1)],
                                 xm[:, 512 * k:512 * (k + 1)],
                                 start=(k == 4), stop=(k == 7))
            ym2 = wp.tile([128, 512], F32, name="ym2")
            nc.scalar.copy(ym2[:], pm2a[:])
            nc.vector.tensor_add(ym2[:], ym2[:], pm2b[:])
            mst2 = wp.tile([128, 6], F32, name="mst2")
            nc.vector.bn_stats(mst2[:], ym2[:])
            b2i = dp.tile([128, 6], F32, name="b2i")
            nc.sync.dma_start(b2i[:], mst2[:])
            b2o = dp.tile([1024, 6], F32, name="b2o")
            nc.gpsimd.collective_compute("AllGather", OP.bypass, replica_groups=RG,
                                         ins=[b2i.opt()], outs=[b2o.opt()])
            keep_warm(22)
            gst2 = wp.tile([128, 48], F32, name="gst2")
            nc.sync.dma_start(gst2[:].rearrange("p (r s) -> p r s", r=8),
                              b2o[:].rearrange("(r p) s -> p r s", p=128))
            mv2 = wp.tile([128, 2], F32, name="mv2")
            nc.vector.bn_aggr(mv2[:], gst2[:].rearrange("p (r t e) -> p r t e",
                                                        r=8, e=3))
            rstd2m = _nr_rsqrt(nc, wp, mv2[:, 1:2], 1.0, "rm2")
            s2m = wp.tile([128, 1], F32, name="s2m")
            nc.vector.tensor_mul(s2m[:], rstd2m[:], gm2S[:])
            t2m = wp.tile([128, 1], F32, name="t2m")
            nc.vector.tensor_mul(t2m[:], mv2[:, 0:1], s2m[:])
            b2m = wp.tile([128, 1], F32, name="b2m")
            nc.vector.tensor_sub(b2m[:], bm2S[:], t2m[:])
            nc.scalar.activation(hT[:], ym2[:], AF.Relu,
                                 bias=b2m[:, 0:1], scale=s2m[:, 0:1])

        nc.sync.dma_start(d["h_out"].ap(), hT[:])


def _build():
    key = (T_STEPS, SIG_MODE)
    if key in _CACHE:
        return _CACHE[key]
    nc = bacc.Bacc("TRN2", target_bir_lowering=False, debug=False,
                   enable_asserts=True, num_devices=NCORES)
    d = {}
    ins = [
        ("hT", (128, 512)), ("cT", (128, 512)), ("posT", (2, 512)),
        ("relposT", (2, 512)),
        ("Wih", (64, 512)), ("Whh", (128, 512)), ("Wse", (2, 64)),
        ("Whp", (128, 2)), ("Wpse", (2, 64)), ("Wp1a", (64, 512)),
        ("Wp1b", (128, 512)), ("Wp2r", (128, 512)), ("Wm1r", (128, 2048)),
        ("Wm2r", (128, 1024)),
        ("bgS", (128, 4)), ("bgnS", (128, 4)), ("bg2S", (128, 4)),
        ("bseS", (64, 1)), ("bhpS", (2, 1)),
        ("gp1S", (128, 4)), ("bp1S", (128, 4)), ("gp2S", (128, 1)),
        ("bp2S", (128, 1)), ("gm1S", (128, 8)), ("bm1S", (128, 8)),
        ("gm2S", (128, 1)), ("bm2S", (128, 1)),
    ]
    for name, shape in ins:
        d[name] = nc.dram_tensor(name, list(shape), F32, kind="ExternalInput")
    d["traj"] = nc.dram_tensor("traj", [T_STEPS, 2, 512], F32, kind="ExternalOutput")
    d["h_out"] = nc.dram_tensor("h_out", [128, 512], F32, kind="ExternalOutput")
    with tile.TileContext(nc) as tc:
        _emit(nc, tc, d)
    nc.compile()
    _CACHE[key] = nc
    return nc


def _prep_inputs(inputs):
    f = lambda x: np.ascontiguousarray(np.asarray(x), dtype=np.float32)
    W_se, b_se = f(inputs["W_se"]), f(inputs["b_se"])
    W_hp, b_hp = f(inputs["W_hp"]), f(inputs["b_hp"])
    Wih, Whh = f(inputs["Wih"]), f(inputs["Whh"])
    bg = f(inputs["bih"]) + f(inputs["bhh"])
    Wp_se = f(inputs["Wp_se"])
    Wp1, Wp2 = f(inputs["Wp1"]), f(inputs["Wp2"])
    Wm1, Wm2 = f(inputs["Wm1"]), f(inputs["Wm2"])
    h0, c0 = f(inputs["h0"]), f(inputs["c0"])
    last_pos, last_pos_rel = f(inputs["last_pos"]), f(inputs["last_pos_rel"])

    col = lambda x: np.ascontiguousarray(x.reshape(-1, 1), np.float32)
    chunks = lambda x, n: np.ascontiguousarray(x.reshape(n, 128).T, np.float32)

    shared = {
        "Wih": Wih, "Whh": Whh, "Wse": W_se, "Whp": W_hp, "Wpse": Wp_se,
        "Wp1a": np.ascontiguousarray(Wp1[:64]),
        "Wp1b": np.ascontiguousarray(Wp1[64:]),
        "Wp2r": np.ascontiguousarray(
            Wp2.reshape(4, 128, 128).transpose(1, 0, 2).reshape(128, 512)),
        "Wm1r": np.ascontiguousarray(
            np.concatenate([Wm1[:128], Wm1[128:]], axis=1)),
        "Wm2r": np.ascontiguousarray(
            Wm2.reshape(8, 128, 128).transpose(1, 0, 2).reshape(128, 1024)),
        "bgS": chunks(bg, 4), "bgnS": chunks(-bg, 4), "bg2S": chunks(2 * bg, 4),
        "bseS": col(b_se), "bhpS": col(b_hp),
        "gp1S": chunks(f(inputs["gp1"]), 4), "bp1S": chunks(f(inputs["betap1"]), 4),
        "gp2S": col(f(inputs["gp2"])), "bp2S": col(f(inputs["betap2"])),
        "gm1S": chunks(f(inputs["gm1"]), 8), "bm1S": chunks(f(inputs["betam1"]), 8),
        "gm2S": col(f(inputs["gm2"])), "bm2S": col(f(inputs["betam2"])),
    }
    in_maps = []
    for i in range(NCORES):
        sl = slice(R * i, R * (i + 1))
        m = dict(shared)
        m["hT"] = np.ascontiguousarray(h0[0, sl].T)
        m["cT"] = np.ascontiguousarray(c0[0, sl].T)
        m["posT"] = np.ascontiguousarray(last_pos[sl].T)
        m["relposT"] = np.ascontiguousarray(last_pos_rel[sl].T)
        in_maps.append(m)
    return in_maps


def run_compiled(inputs, trace=False, **kw):
    nc = _build()
    in_maps = _prep_inputs(inputs)
    res = bass_utils.run_bass_kernel_spmd(nc, in_maps,
                                          core_ids=list(range(NCORES)),
                                          trace=trace, **kw)
    traj = np.empty((T_STEPS, NCORES * R, 2), np.float32)
    h = np.empty((1, NCORES * R, 128), np.float32)
    for i in range(NCORES):
        traj[:, R * i:R * (i + 1), :] = res.results[i]["traj"].transpose(0, 2, 1)
        h[0, R * i:R * (i + 1), :] = res.results[i]["h_out"].T
    return (traj, h), res


def kernel(**inputs):
    out, _ = run_compiled(inputs, trace=False)
    return out
